# revision 1
# baseline (speedup 1.0000x reference)
"""Trainium2 Bass kernel for nn_EquivariantDecoder.

Data-parallel over 8 NeuronCores (batch sharded). Per core:
  - v rows are DMA'd contiguously (SWDGE fp32->bf16 cast),
  - PE strided transposes build per-irrep [i, (m,b)] tiles,
  - e3linear layers run as per-(l,m) bf16 matmuls packed into 32x32
    PE sub-arrays (tile_position), gates as DVE tensor_tensor with the
    sigmoid tiles partition-aligned to the matmul output banks,
  - layer 4 accumulates all 49 output channels into one PSUM bank,
    which is transposed back to [b, 49] and DMA'd out.
"""

import numpy as np
import ml_dtypes
from contextlib import ExitStack

import concourse.bass as bass
import concourse.mybir as mybir
import concourse.tile as tile
from concourse import bass_utils

BF16 = mybir.dt.bfloat16
FP32 = mybir.dt.float32

# ---------------- problem constants (hardcoded) ----------------
B_FULL = 16384
NCORES = 8
BC = B_FULL // NCORES          # 2048 rows per core
BT = 512                       # b-tile

IN_IRREPS = [(256, 0), (128, 1), (128, 2), (64, 3), (64, 4), (64, 5), (64, 6)]
HID_IRREPS = [(64, 0), (64, 1), (64, 2), (32, 3), (32, 4), (32, 5), (32, 6)]
N_SCALARS = 64
N_GATES = 256
D_IN = 3840
D_OUT = 49

# feature offsets of IN_IRREPS blocks in v rows
IN_OFF = {}
_o = 0
for _mul, _l in IN_IRREPS:
    IN_OFF[_l] = _o
    _o += _mul * (2 * _l + 1)

# output channel offsets (l-blocks of the 49-dim output)
OUT_OFF = {l: l * l for l in range(7)}  # 0,1,4,9,16,25,36

_BUILD = {}


def _pack_weights(w1, w2, w3, w4):
    """Host-side packing of the flat e3nn weight vectors into the SBUF
    layouts the kernel loads. Returns dict of np arrays (bf16/fp32)."""
    bf = ml_dtypes.bfloat16
    out = {}

    def split_blocks(wflat, in_irr, out_irr):
        mul_in = {l: m for m, l in in_irr}
        blocks = []
        off = 0
        for mo, l in out_irr:
            mi = mul_in[l]
            w = wflat[off:off + mi * mo].reshape(mi, mo) / np.sqrt(mi)
            off += mi * mo
            blocks.append((l, w))
        assert off == wflat.size
        return blocks

    pre_irr = [(N_SCALARS, 0), (N_GATES, 0)] + [(m, l) for m, l in HID_IRREPS if l > 0]

    # Gate/hidden partition layouts use DESCENDING l so that for every m
    # the valid channels form a partition PREFIX (l=6 highest m count first):
    #   A tiles: [l2 (0:64) | l1 (64:128)]
    #   B tiles: [l6 (0:32) | l5 | l4 | l3 (96:128)]
    # This lets layer-4 contract prefixes at base partition 0 (single PE
    # row-group chain -> safe PSUM accumulation) with no garbage reads.

    # ---- layer 1 ----
    b1 = split_blocks(w1, IN_IRREPS, pre_irr)
    ws, wg = b1[0][1], b1[1][1]                      # [256,64], [256,256]
    # gate channel order in wg: [g1(64) g2(64) g3(32) g4 g5 g6]
    gperm = ([64 + i for i in range(64)] + [i for i in range(64)] +
             [224 + i for i in range(32)] + [192 + i for i in range(32)] +
             [160 + i for i in range(32)] + [128 + i for i in range(32)])
    wg = wg[:, gperm]                                # [gA: g2|g1, gB: g6|g5|g4|g3]
    W10 = np.concatenate([ws, wg], axis=1)           # [256, 320]
    out["W1_0a"] = W10[:128].astype(bf)
    out["W1_0b"] = W10[128:].astype(bf)
    w1l = {l: w for l, w in b1[2:]}
    out["W1_l1"] = w1l[1].astype(bf)                 # [128, 64]
    out["W1_l2"] = w1l[2].astype(bf)                 # [128, 64]
    out["W1_b34"] = np.concatenate([w1l[3], w1l[4]], axis=0).astype(bf)  # [128,32]
    out["W1_b56"] = np.concatenate([w1l[5], w1l[6]], axis=0).astype(bf)  # [128,32]

    # ---- layers 2, 3 ----
    for name, wflat in (("W2", w2), ("W3", w3)):
        b = split_blocks(wflat, HID_IRREPS, pre_irr)
        ws, wg = b[0][1], b[1][1]                    # [64,64], [64,256]
        wg = wg[:, gperm]
        out[name + "_0"] = np.concatenate([ws, wg], axis=1).astype(bf)   # [64, 320]
        wl = {l: w for l, w in b[2:]}
        # rows follow the h-tile (descending-l) layout of the INPUT
        out[name + "_A"] = np.concatenate([wl[2], wl[1]], axis=0).astype(bf)  # [128,64]
        out[name + "_B"] = np.concatenate([wl[6], wl[5], wl[4], wl[3]], axis=0).astype(bf)  # [128,32]

    # ---- layer 4: per-m column matrices accumulating into [49] ----
    b4 = split_blocks(w4, HID_IRREPS, [(1, l) for l in range(7)])
    w4l = {l: w[:, 0] for l, w in b4}
    W4B = np.zeros((128, 13, D_OUT), np.float32)
    for l in (3, 4, 5, 6):
        pd = 32 * (6 - l)
        for m in range(2 * l + 1):
            W4B[pd:pd + 32, m, OUT_OFF[l] + m] = w4l[l]
    out["W4_B"] = W4B.astype(bf)
    W4A = np.zeros((128, 5, D_OUT), np.float32)
    for m in range(5):
        W4A[0:64, m, OUT_OFF[2] + m] = w4l[2]
    for m in range(3):
        W4A[64:128, m, OUT_OFF[1] + m] = w4l[1]
    out["W4_A"] = W4A.astype(bf)
    W40 = np.zeros((64, D_OUT), np.float32)
    W40[:, 0] = w4l[0]
    out["W4_0"] = W40.astype(bf)

    out["ident"] = np.eye(128, dtype=bf)
    out["ident49"] = np.eye(D_OUT, dtype=np.float32)
    return out


def _split_excess_waits(nc, max_waits=1):
    """This walrus build accepts only one sem-wait per instruction on
    some ops; hoist excess waits onto same-engine NoOps inserted before."""
    for f in nc.m.functions:
        for bb in f.blocks:
            newlist = []
            changed = False
            for ins in bb.instructions:
                si = ins.sync_info
                waits = list(si.on_wait) if (si and si.on_wait) else []
                if len(waits) > max_waits:
                    extras, keep = waits[:-max_waits], waits[-max_waits:]
                    for k in range(0, len(extras), max_waits):
                        nop = mybir.InstNoOp(
                            name=f"{ins.name}_waitnop{k}", ins=[], outs=[],
                            engine=ins.engine)
                        nop.sync_info = mybir.SyncInfo(
                            on_wait=extras[k:k + max_waits], on_update=[])
                        nc.register_instruction(nop)
                        newlist.append(nop)
                    ins.sync_info = mybir.SyncInfo(
                        on_wait=keep,
                        on_update=list(si.on_update) if si.on_update else [])
                    changed = True
                newlist.append(ins)
            if changed:
                bb.instructions[:] = newlist
    return nc


def _build_program(BC=BC, BT=BT, stages="T1234"):
    NT = BC // BT
    NR = BT // 128
    nc = bass.Bass("TRN2", target_bir_lowering=False, debug=False)

    v = nc.dram_tensor("v", [BC, D_IN], FP32, kind="ExternalInput").ap()
    wd = {}
    wspecs = [
        ("W1_0a", [128, 320]), ("W1_0b", [128, 320]),
        ("W1_l1", [128, 64]), ("W1_l2", [128, 64]),
        ("W1_b34", [128, 32]), ("W1_b56", [128, 32]),
        ("W2_0", [64, 320]), ("W2_A", [128, 64]), ("W2_B", [128, 32]),
        ("W3_0", [64, 320]), ("W3_A", [128, 64]), ("W3_B", [128, 32]),
        ("W4_B", [128, 13, D_OUT]), ("W4_A", [128, 5, D_OUT]), ("W4_0", [64, D_OUT]),
        ("ident", [128, 128]),
    ]
    for name, shape in wspecs:
        wd[name] = nc.dram_tensor(name, shape, BF16, kind="ExternalInput").ap()
    wd["ident49"] = nc.dram_tensor("ident49", [D_OUT, D_OUT], FP32,
                                   kind="ExternalInput").ap()
    out = nc.dram_tensor("out", [BC, D_OUT], FP32, kind="ExternalOutput").ap()

    with tile.TileContext(nc) as tc:
        with ExitStack() as ctx:
            _emit(ctx, tc, nc, v, wd, out, BC, BT, NT, NR, stages)

    _split_excess_waits(nc)
    return nc


def _emit(ctx, tc, nc, v, wd, out, BC, BT, NT, NR, stages):
    mm = nc.tensor.matmul
    Mult = mybir.AluOpType.mult

    wpool = ctx.enter_context(tc.tile_pool(name="weights", bufs=1))
    rows_pool = ctx.enter_context(tc.tile_pool(name="rows", bufs=3))
    vpool = ctx.enter_context(tc.tile_pool(name="vtiles", bufs=2))
    hpool = ctx.enter_context(tc.tile_pool(name="htiles", bufs=2))
    gpool = ctx.enter_context(tc.tile_pool(name="gates", bufs=2))
    opool = ctx.enter_context(tc.tile_pool(name="outs", bufs=2))
    tppool = ctx.enter_context(tc.tile_pool(name="tp", bufs=2, space="PSUM"))
    zpool = ctx.enter_context(tc.tile_pool(name="zb", bufs=1, space="PSUM"))
    z4pool = ctx.enter_context(tc.tile_pool(name="z4", bufs=1, space="PSUM"))

    # ---- load weights / identity once ----
    W = {}
    for name in ("W1_0a", "W1_0b", "W1_l1", "W1_l2", "W1_b34", "W1_b56",
                 "W2_0", "W2_A", "W2_B", "W3_0", "W3_A", "W3_B",
                 "W4_B", "W4_A", "W4_0", "ident", "ident49"):
        t = wpool.tile(list(wd[name].shape), wd[name].dtype, tag=name)
        nc.sync.dma_start(out=t, in_=wd[name])
        W[name] = t

    ident = W["ident"]

    # transpose source views of one [128, 3840] row-chunk
    def lblock_view(rows, l):
        mul = dict((ll, m) for m, ll in IN_IRREPS)[l]
        d = 2 * l + 1
        o = IN_OFF[l]
        return rows[:, o:o + mul * d].rearrange("p (i m) -> p i m", m=d)

    for t in range(NT):
        # ---------------- input stage: rows -> v-tiles ----------------
        v0 = vpool.tile([128, 2, BT], BF16, tag="v0")
        vA1 = vpool.tile([128, 3, BT], BF16, tag="vA1")
        vA2 = vpool.tile([128, 5, BT], BF16, tag="vA2")
        vB34 = vpool.tile([128, 9, BT], BF16, tag="vB34")
        vB56 = vpool.tile([128, 13, BT], BF16, tag="vB56")

        for r in range(NR if "T" in stages else 0):
            rows = rows_pool.tile([128, D_IN], BF16, tag="rows")
            nc.gpsimd.dma_start(out=rows, in_=v[t * BT + r * 128: t * BT + (r + 1) * 128, :])
            rb = slice(r * 128, (r + 1) * 128)

            # l=0: two contiguous 128-col transposes
            tp0 = tppool.tile([128, 2, 128], BF16, tag="tp")
            nc.tensor.transpose(tp0[:, 0, :], rows[:, 0:128], ident)
            nc.tensor.transpose(tp0[:, 1, :], rows[:, 128:256], ident)
            nc.scalar.copy(out=v0[:, :, rb], in_=tp0)

            # l=1 (3 slots) + l=2 (5 slots), all full-partition
            tpA = tppool.tile([128, 8, 128], BF16, tag="tp")
            r1 = lblock_view(rows, 1)
            r2 = lblock_view(rows, 2)
            for m in range(3):
                nc.tensor.transpose(tpA[:, m, :], r1[:, :, m], ident)
            for m in range(5):
                nc.tensor.transpose(tpA[:, 3 + m, :], r2[:, :, m], ident)
            nc.scalar.copy(out=vA1[:, :, rb], in_=tpA[:, 0:3, :])
            nc.scalar.copy(out=vA2[:, :, rb], in_=tpA[:, 3:8, :])

            # l=3 (7 slots, rows 0:64) + l=4 (9 slots, rows 64:128)
            tp34 = tppool.tile([128, 9, 128], BF16, tag="tp")
            r3 = lblock_view(rows, 3)
            r4 = lblock_view(rows, 4)
            for m in range(7):
                nc.tensor.transpose(tp34[0:64, m, :], r3[:, :, m], ident,
                                    tile_position=(0, 0))
            for m in range(9):
                nc.tensor.transpose(tp34[64:128, m, :], r4[:, :, m], ident,
                                    tile_position=(0, 64))
            nc.scalar.copy(out=vB34[:, 0:7, rb], in_=tp34[:, 0:7, :])
            nc.scalar.copy(out=vB34[64:128, 7:9, rb], in_=tp34[64:128, 7:9, :])

            # l=5 (11 slots, rows 0:64) + l=6 (13 slots, rows 64:128)
            tp56 = tppool.tile([128, 13, 128], BF16, tag="tp")
            r5 = lblock_view(rows, 5)
            r6 = lblock_view(rows, 6)
            for m in range(11):
                nc.tensor.transpose(tp56[0:64, m, :], r5[:, :, m], ident,
                                    tile_position=(0, 0))
            for m in range(13):
                nc.tensor.transpose(tp56[64:128, m, :], r6[:, :, m], ident,
                                    tile_position=(0, 64))
            nc.scalar.copy(out=vB56[:, 0:11, rb], in_=tp56[:, 0:11, :])
            nc.scalar.copy(out=vB56[64:128, 11:13, rb], in_=tp56[64:128, 11:13, :])

        # ---------------- the three gated e3linear layers ----------------
        def layer(w0, wA, wB, x0, xA, xB, first):
            """x0 [<=128, BT] (l0 feats, 1 or 2 k-slots), xA [128, mA, BT],
            xB [128, mB, BT] -> new (h0, hA, hB)."""
            h0 = hpool.tile([64, BT], BF16, tag="h0")
            hA = hpool.tile([128, 5, BT], BF16, tag="hA")
            hB = hpool.tile([128, 13, BT], BF16, tag="hB")
            gA = gpool.tile([128, BT], BF16, tag="gA")
            gB = gpool.tile([128, BT], BF16, tag="gB")
            sig = gpool.tile([64, BT], BF16, tag="sig")

            # scalars+gates head: z0 slots [s | gA | gB]
            z0 = zpool.tile([128, 3, BT], FP32, tag="zb")
            if first:
                for k, wk in enumerate((W["W1_0a"], W["W1_0b"])):
                    st, sp = (k == 0), (k == 1)
                    mm(z0[0:64, 0, :], wk[:, 0:64], x0[:, k, :], start=st, stop=sp)
                    mm(z0[:, 1, :], wk[:, 64:192], x0[:, k, :], start=st, stop=sp)
                    mm(z0[:, 2, :], wk[:, 192:320], x0[:, k, :], start=st, stop=sp)
            else:
                mm(z0[0:64, 0, :], w0[:, 0:64], x0, start=True, stop=True)
                mm(z0[:, 1, :], w0[:, 64:192], x0, start=True, stop=True)
                mm(z0[:, 2, :], w0[:, 192:320], x0, start=True, stop=True)
            nc.scalar.activation(gA, z0[:, 1, :], mybir.ActivationFunctionType.Sigmoid)
            nc.scalar.activation(gB, z0[:, 2, :], mybir.ActivationFunctionType.Sigmoid)
            nc.scalar.activation(sig, z0[0:64, 0, :], mybir.ActivationFunctionType.Sigmoid)
            nc.vector.tensor_mul(h0, z0[0:64, 0, :], sig)  # silu

            # A part (descending l): l=2 at rows 0:64, l=1 at rows 64:128
            if first:
                wl1, wl2 = W["W1_l1"], W["W1_l2"]
                a1 = lambda m: xA[0][:, m, :]
                a2 = lambda m: xA[1][:, m, :]
                ap1 = ap2 = 0
            else:
                wl2, wl1 = wA[0:64, :], wA[64:128, :]
                a2 = lambda m: xA[0:64, m, :]
                a1 = lambda m: xA[64:128, m, :]
                ap2, ap1 = 0, 64
            for mlo in (0, 3):
                nm = 3 if mlo == 0 else 2
                zA = zpool.tile([128, 3, BT], FP32, tag="zb")
                for j in range(nm):
                    m = mlo + j
                    mm(zA[0:64, j, :], wl2, a2(m), start=True, stop=True,
                       tile_position=(ap2, 0))
                    if m < 3:
                        mm(zA[64:128, j, :], wl1, a1(m), start=True, stop=True,
                           tile_position=(ap1, 64))
                if mlo == 0:
                    nc.vector.tensor_mul(
                        hA[:, 0:3, :], zA,
                        gA[:, :].unsqueeze(1).broadcast_to([128, 3, BT]))
                else:
                    nc.vector.tensor_mul(
                        hA[0:64, 3:5, :], zA[0:64, 0:2, :],
                        gA[0:64, :].unsqueeze(1).broadcast_to([64, 2, BT]))

            # B part (descending l): l=6 rows 0:32, l=5, l=4, l=3 rows 96:128
            if first:
                wof = {3: (W["W1_b34"], 0), 4: (W["W1_b34"], 64),
                       5: (W["W1_b56"], 0), 6: (W["W1_b56"], 64)}
                bsrc = {3: (xB[0], 0), 4: (xB[0], 64), 5: (xB[1], 0), 6: (xB[1], 64)}
                kw = 64
            else:
                wof = {l: (wB, 32 * (6 - l)) for l in (3, 4, 5, 6)}
                bsrc = {l: (xB, 32 * (6 - l)) for l in (3, 4, 5, 6)}
                kw = 32
            for mlo in (0, 3, 6, 9, 12):
                nm = min(3, 13 - mlo)
                zB = zpool.tile([128, 3, BT], FP32, tag="zb")
                phis = []
                for j in range(nm):
                    m = mlo + j
                    phi = 0
                    for l in (6, 5, 4, 3):
                        if m >= 2 * l + 1:
                            continue
                        wt, wr = wof[l]
                        xs, xr = bsrc[l]
                        pd = 32 * (6 - l)
                        phi = max(phi, pd + 32)
                        mm(zB[pd:pd + 32, j, :], wt[wr:wr + kw, :],
                           xs[xr:xr + kw, m, :], start=True, stop=True,
                           tile_position=(wr if first else xr, pd))
                    phis.append(phi)
                # gate in maximal rectangles of equal valid-partition prefix
                j = 0
                while j < nm:
                    k = j
                    while k < nm and phis[k] == phis[j]:
                        k += 1
                    phi = phis[j]
                    nc.vector.tensor_mul(
                        hB[0:phi, mlo + j:mlo + k, :], zB[0:phi, j:k, :],
                        gB[0:phi, :].unsqueeze(1).broadcast_to(
                            [phi, k - j, BT]))
                    j = k
            return h0, hA, hB

        if "1" not in stages:
            continue
        h0, hA, hB = layer(None, None, None, v0, (vA1, vA2), (vB34, vB56), True)
        if "2" in stages:
            h0, hA, hB = layer(W["W2_0"], W["W2_A"], W["W2_B"], h0, hA, hB, False)
        if "3" in stages:
            h0, hA, hB = layer(W["W3_0"], W["W3_A"], W["W3_B"], h0, hA, hB, False)
        if "4" not in stages:
            continue

        # ---------------- layer 4: accumulate into [49, BT] ----------------
        # layer 4: every matmul contracts a partition PREFIX (base 0) so the
        # whole 19-matmul accumulation chain shares PE row-group 0 and
        # serializes (concurrent same-address PSUM accumulation faults).
        z4 = z4pool.tile([D_OUT, BT], FP32, tag="z4")
        mm(z4, W["W4_0"], h0, start=True, stop=False, tile_position=(0, 0))
        for m in range(5):
            kp = 128 if m < 3 else 64      # [l2 | l1] prefix
            mm(z4, W["W4_A"][0:kp, m, :], hA[0:kp, m, :], start=False, stop=False,
               tile_position=(0, 0))
        for m in range(13):
            nvalid = sum(1 for l in (6, 5, 4, 3) if m < 2 * l + 1)
            kp = 32 * nvalid               # [l6 | l5 | l4 | l3] prefix
            mm(z4, W["W4_B"][0:kp, m, :], hB[0:kp, m, :],
               start=False, stop=(m == 12), tile_position=(0, 0))

        z4sb = opool.tile([D_OUT, BT], FP32, tag="z4sb")
        nc.vector.tensor_copy(z4sb, z4)
        for r in range(NR):
            z4t = z4pool.tile([128, D_OUT], FP32, tag="z4")
            nc.tensor.transpose(z4t, z4sb[:, r * 128:(r + 1) * 128], W["ident49"])
            outT = opool.tile([128, D_OUT], FP32, tag="outT")
            nc.vector.tensor_copy(outT, z4t)
            nc.sync.dma_start(out=out[t * BT + r * 128: t * BT + (r + 1) * 128, :],
                              in_=outT)


def _get_nc():
    if "nc" not in _BUILD:
        _BUILD["nc"] = _build_program()
    return _BUILD["nc"]


def kernel(v_raw, w1, w2, w3, w4):
    nc = _get_nc()
    wmap = _pack_weights(np.asarray(w1), np.asarray(w2), np.asarray(w3),
                         np.asarray(w4))
    v_raw = np.ascontiguousarray(np.asarray(v_raw, dtype=np.float32))
    in_maps = []
    for c in range(NCORES):
        m = dict(wmap)
        m["v"] = v_raw[c * BC:(c + 1) * BC]
        in_maps.append(m)
    res = bass_utils.run_bass_kernel_spmd(nc, in_maps, core_ids=list(range(NCORES)))
    outs = [res.results[c]["out"] for c in range(NCORES)]
    full = np.concatenate(outs, axis=0)            # [B, 49]
    return full.reshape(B_FULL, D_OUT, 1).astype(np.float32)



# revision 2
# speedup vs baseline: 183.3082x; 183.3082x over previous
"""Trainium2 Bass kernel for nn_EquivariantDecoder.

Data-parallel over 8 NeuronCores (batch sharded). Per core:
  - v rows are DMA'd contiguously (SWDGE fp32->bf16 cast),
  - PE strided transposes build per-irrep [i, (m,b)] tiles,
  - e3linear layers run as per-(l,m) bf16 matmuls packed into 32x32
    PE sub-arrays (tile_position), gates as DVE tensor_tensor with the
    sigmoid tiles partition-aligned to the matmul output banks,
  - layer 4 accumulates all 49 output channels into one PSUM bank,
    which is transposed back to [b, 49] and DMA'd out.
"""

import numpy as np
import ml_dtypes
from contextlib import ExitStack

import concourse.bass as bass
import concourse.mybir as mybir
import concourse.tile as tile
from concourse import bass_utils

BF16 = mybir.dt.bfloat16
FP32 = mybir.dt.float32

# ---------------- problem constants (hardcoded) ----------------
B_FULL = 16384
NCORES = 8
BC = B_FULL // NCORES          # 2048 rows per core
BT = 512                       # b-tile

IN_IRREPS = [(256, 0), (128, 1), (128, 2), (64, 3), (64, 4), (64, 5), (64, 6)]
HID_IRREPS = [(64, 0), (64, 1), (64, 2), (32, 3), (32, 4), (32, 5), (32, 6)]
N_SCALARS = 64
N_GATES = 256
D_IN = 3840
D_OUT = 49

# feature offsets of IN_IRREPS blocks in v rows
IN_OFF = {}
_o = 0
for _mul, _l in IN_IRREPS:
    IN_OFF[_l] = _o
    _o += _mul * (2 * _l + 1)

# output channel offsets (l-blocks of the 49-dim output)
OUT_OFF = {l: l * l for l in range(7)}  # 0,1,4,9,16,25,36

_BUILD = {}


def _pack_weights(w1, w2, w3, w4):
    """Host-side packing of the flat e3nn weight vectors into the SBUF
    layouts the kernel loads. Returns dict of np arrays (bf16/fp32)."""
    bf = ml_dtypes.bfloat16
    out = {}

    def split_blocks(wflat, in_irr, out_irr):
        mul_in = {l: m for m, l in in_irr}
        blocks = []
        off = 0
        for mo, l in out_irr:
            mi = mul_in[l]
            w = wflat[off:off + mi * mo].reshape(mi, mo) / np.sqrt(mi)
            off += mi * mo
            blocks.append((l, w))
        assert off == wflat.size
        return blocks

    pre_irr = [(N_SCALARS, 0), (N_GATES, 0)] + [(m, l) for m, l in HID_IRREPS if l > 0]

    # Gate/hidden partition layouts use DESCENDING l so that for every m
    # the valid channels form a partition PREFIX (l=6 highest m count first):
    #   A tiles: [l2 (0:64) | l1 (64:128)]
    #   B tiles: [l6 (0:32) | l5 | l4 | l3 (96:128)]
    # This lets layer-4 contract prefixes at base partition 0 (single PE
    # row-group chain -> safe PSUM accumulation) with no garbage reads.

    # ---- layer 1 ----
    b1 = split_blocks(w1, IN_IRREPS, pre_irr)
    ws, wg = b1[0][1], b1[1][1]                      # [256,64], [256,256]
    # gate channel order in wg: [g1(64) g2(64) g3(32) g4 g5 g6]
    gperm = ([64 + i for i in range(64)] + [i for i in range(64)] +
             [224 + i for i in range(32)] + [192 + i for i in range(32)] +
             [160 + i for i in range(32)] + [128 + i for i in range(32)])
    wg = wg[:, gperm]                                # [gA: g2|g1, gB: g6|g5|g4|g3]
    W10 = np.concatenate([ws, wg], axis=1)           # [256, 320]
    out["W1_0a"] = W10[:128].astype(bf)
    out["W1_0b"] = W10[128:].astype(bf)
    w1l = {l: w for l, w in b1[2:]}
    out["W1_l1"] = w1l[1].astype(bf)                 # [128, 64]
    out["W1_l2"] = w1l[2].astype(bf)                 # [128, 64]
    out["W1_b34"] = np.concatenate([w1l[3], w1l[4]], axis=0).astype(bf)  # [128,32]
    out["W1_b56"] = np.concatenate([w1l[5], w1l[6]], axis=0).astype(bf)  # [128,32]

    # ---- layers 2, 3 ----
    for name, wflat in (("W2", w2), ("W3", w3)):
        b = split_blocks(wflat, HID_IRREPS, pre_irr)
        ws, wg = b[0][1], b[1][1]                    # [64,64], [64,256]
        wg = wg[:, gperm]
        out[name + "_0"] = np.concatenate([ws, wg], axis=1).astype(bf)   # [64, 320]
        wl = {l: w for l, w in b[2:]}
        # rows follow the h-tile (descending-l) layout of the INPUT
        out[name + "_A"] = np.concatenate([wl[2], wl[1]], axis=0).astype(bf)  # [128,64]
        out[name + "_B"] = np.concatenate([wl[6], wl[5], wl[4], wl[3]], axis=0).astype(bf)  # [128,32]

    # ---- layer 4: per-m column matrices accumulating into [49] ----
    b4 = split_blocks(w4, HID_IRREPS, [(1, l) for l in range(7)])
    w4l = {l: w[:, 0] for l, w in b4}
    W4B = np.zeros((128, 13, D_OUT), np.float32)
    for l in (3, 4, 5, 6):
        pd = 32 * (6 - l)
        for m in range(2 * l + 1):
            W4B[pd:pd + 32, m, OUT_OFF[l] + m] = w4l[l]
    out["W4_B"] = W4B.astype(bf)
    W4A = np.zeros((128, 5, D_OUT), np.float32)
    for m in range(5):
        W4A[0:64, m, OUT_OFF[2] + m] = w4l[2]
    for m in range(3):
        W4A[64:128, m, OUT_OFF[1] + m] = w4l[1]
    out["W4_A"] = W4A.astype(bf)
    W40 = np.zeros((64, D_OUT), np.float32)
    W40[:, 0] = w4l[0]
    out["W4_0"] = W40.astype(bf)

    out["ident"] = np.eye(128, dtype=bf)
    out["ident49"] = np.eye(D_OUT, dtype=np.float32)
    return out


def _split_excess_waits(nc, max_waits=1):
    """This walrus build accepts only one sem-wait per instruction on
    some ops; hoist excess waits onto same-engine NoOps inserted before."""
    for f in nc.m.functions:
        for bb in f.blocks:
            newlist = []
            changed = False
            for ins in bb.instructions:
                si = ins.sync_info
                waits = list(si.on_wait) if (si and si.on_wait) else []
                if len(waits) > max_waits:
                    extras, keep = waits[:-max_waits], waits[-max_waits:]
                    for k in range(0, len(extras), max_waits):
                        nop = mybir.InstNoOp(
                            name=f"{ins.name}_waitnop{k}", ins=[], outs=[],
                            engine=ins.engine)
                        nop.sync_info = mybir.SyncInfo(
                            on_wait=extras[k:k + max_waits], on_update=[])
                        nc.register_instruction(nop)
                        newlist.append(nop)
                    ins.sync_info = mybir.SyncInfo(
                        on_wait=keep,
                        on_update=list(si.on_update) if si.on_update else [])
                    changed = True
                newlist.append(ins)
            if changed:
                bb.instructions[:] = newlist
    return nc


def _build_program(BC=BC, BT=BT, stages="T1234"):
    NT = BC // BT
    NR = BT // 128
    nc = bass.Bass("TRN2", target_bir_lowering=False, debug=False)

    v = nc.dram_tensor("v", [BC, D_IN], FP32, kind="ExternalInput").ap()
    wd = {}
    wspecs = [
        ("W1_0a", [128, 320]), ("W1_0b", [128, 320]),
        ("W1_l1", [128, 64]), ("W1_l2", [128, 64]),
        ("W1_b34", [128, 32]), ("W1_b56", [128, 32]),
        ("W2_0", [64, 320]), ("W2_A", [128, 64]), ("W2_B", [128, 32]),
        ("W3_0", [64, 320]), ("W3_A", [128, 64]), ("W3_B", [128, 32]),
        ("W4_B", [128, 13, D_OUT]), ("W4_A", [128, 5, D_OUT]), ("W4_0", [64, D_OUT]),
        ("ident", [128, 128]),
    ]
    for name, shape in wspecs:
        wd[name] = nc.dram_tensor(name, shape, BF16, kind="ExternalInput").ap()
    wd["ident49"] = nc.dram_tensor("ident49", [D_OUT, D_OUT], FP32,
                                   kind="ExternalInput").ap()
    out = nc.dram_tensor("out", [BC, D_OUT], FP32, kind="ExternalOutput").ap()

    with tile.TileContext(nc) as tc:
        with ExitStack() as ctx:
            _emit(ctx, tc, nc, v, wd, out, BC, BT, NT, NR, stages)

    _split_excess_waits(nc)
    return nc


def _emit(ctx, tc, nc, v, wd, out, BC, BT, NT, NR, stages):
    mm = nc.tensor.matmul
    Mult = mybir.AluOpType.mult

    wpool = ctx.enter_context(tc.tile_pool(name="weights", bufs=1))
    rows_pool = ctx.enter_context(tc.tile_pool(name="rows", bufs=3))
    vpool = ctx.enter_context(tc.tile_pool(name="vtiles", bufs=2))
    hpool = ctx.enter_context(tc.tile_pool(name="htiles", bufs=2))
    gpool = ctx.enter_context(tc.tile_pool(name="gates", bufs=2))
    opool = ctx.enter_context(tc.tile_pool(name="outs", bufs=2))
    tppool = ctx.enter_context(tc.tile_pool(name="tp", bufs=2, space="PSUM"))
    zpool = ctx.enter_context(tc.tile_pool(name="zb", bufs=1, space="PSUM"))
    z4pool = ctx.enter_context(tc.tile_pool(name="z4", bufs=1, space="PSUM"))

    # ---- load weights / identity once ----
    W = {}
    for name in ("W1_0a", "W1_0b", "W1_l1", "W1_l2", "W1_b34", "W1_b56",
                 "W2_0", "W2_A", "W2_B", "W3_0", "W3_A", "W3_B",
                 "W4_B", "W4_A", "W4_0", "ident", "ident49"):
        t = wpool.tile(list(wd[name].shape), wd[name].dtype, tag=name)
        nc.sync.dma_start(out=t, in_=wd[name])
        W[name] = t

    ident = W["ident"]

    # transpose source views of one [128, 3840] row-chunk
    def lblock_view(rows, l):
        mul = dict((ll, m) for m, ll in IN_IRREPS)[l]
        d = 2 * l + 1
        o = IN_OFF[l]
        return rows[:, o:o + mul * d].rearrange("p (i m) -> p i m", m=d)

    for t in range(NT):
        # ---------------- input stage: rows -> v-tiles ----------------
        v0 = vpool.tile([128, 2, BT], BF16, tag="v0")
        vA1 = vpool.tile([128, 3, BT], BF16, tag="vA1")
        vA2 = vpool.tile([128, 5, BT], BF16, tag="vA2")
        vB34 = vpool.tile([128, 9, BT], BF16, tag="vB34")
        vB56 = vpool.tile([128, 13, BT], BF16, tag="vB56")

        for r in range(NR if "T" in stages else 0):
            rows = rows_pool.tile([128, D_IN], BF16, tag="rows")
            nc.gpsimd.dma_start(out=rows, in_=v[t * BT + r * 128: t * BT + (r + 1) * 128, :])
            rb = slice(r * 128, (r + 1) * 128)

            # l=0: two contiguous 128-col transposes
            tp0 = tppool.tile([128, 2, 128], BF16, tag="tp")
            nc.tensor.transpose(tp0[:, 0, :], rows[:, 0:128], ident)
            nc.tensor.transpose(tp0[:, 1, :], rows[:, 128:256], ident)
            nc.scalar.copy(out=v0[:, :, rb], in_=tp0)

            # l=1 (3 slots) + l=2 (5 slots), all full-partition
            tpA = tppool.tile([128, 8, 128], BF16, tag="tp")
            r1 = lblock_view(rows, 1)
            r2 = lblock_view(rows, 2)
            for m in range(3):
                nc.tensor.transpose(tpA[:, m, :], r1[:, :, m], ident)
            for m in range(5):
                nc.tensor.transpose(tpA[:, 3 + m, :], r2[:, :, m], ident)
            nc.scalar.copy(out=vA1[:, :, rb], in_=tpA[:, 0:3, :])
            nc.scalar.copy(out=vA2[:, :, rb], in_=tpA[:, 3:8, :])

            # l=3 (7 slots, rows 0:64) + l=4 (9 slots, rows 64:128)
            tp34 = tppool.tile([128, 9, 128], BF16, tag="tp")
            r3 = lblock_view(rows, 3)
            r4 = lblock_view(rows, 4)
            for m in range(7):
                nc.tensor.transpose(tp34[0:64, m, :], r3[:, :, m], ident,
                                    tile_position=(0, 0))
            for m in range(9):
                nc.tensor.transpose(tp34[64:128, m, :], r4[:, :, m], ident,
                                    tile_position=(0, 64))
            nc.scalar.copy(out=vB34[:, 0:7, rb], in_=tp34[:, 0:7, :])
            nc.scalar.copy(out=vB34[64:128, 7:9, rb], in_=tp34[64:128, 7:9, :])

            # l=5 (11 slots, rows 0:64) + l=6 (13 slots, rows 64:128)
            tp56 = tppool.tile([128, 13, 128], BF16, tag="tp")
            r5 = lblock_view(rows, 5)
            r6 = lblock_view(rows, 6)
            for m in range(11):
                nc.tensor.transpose(tp56[0:64, m, :], r5[:, :, m], ident,
                                    tile_position=(0, 0))
            for m in range(13):
                nc.tensor.transpose(tp56[64:128, m, :], r6[:, :, m], ident,
                                    tile_position=(0, 64))
            nc.scalar.copy(out=vB56[:, 0:11, rb], in_=tp56[:, 0:11, :])
            nc.scalar.copy(out=vB56[64:128, 11:13, rb], in_=tp56[64:128, 11:13, :])

        # ---------------- the three gated e3linear layers ----------------
        def layer(w0, wA, wB, x0, xA, xB, first):
            """x0 [<=128, BT] (l0 feats, 1 or 2 k-slots), xA [128, mA, BT],
            xB [128, mB, BT] -> new (h0, hA, hB)."""
            h0 = hpool.tile([64, BT], BF16, tag="h0")
            hA = hpool.tile([128, 5, BT], BF16, tag="hA")
            hB = hpool.tile([128, 13, BT], BF16, tag="hB")
            gA = gpool.tile([128, BT], BF16, tag="gA")
            gB = gpool.tile([128, BT], BF16, tag="gB")
            sig = gpool.tile([64, BT], BF16, tag="sig")

            # scalars+gates head: z0 slots [s | gA | gB]
            z0 = zpool.tile([128, 3, BT], FP32, tag="zb")
            if first:
                for k, wk in enumerate((W["W1_0a"], W["W1_0b"])):
                    st, sp = (k == 0), (k == 1)
                    mm(z0[0:64, 0, :], wk[:, 0:64], x0[:, k, :], start=st, stop=sp)
                    mm(z0[:, 1, :], wk[:, 64:192], x0[:, k, :], start=st, stop=sp)
                    mm(z0[:, 2, :], wk[:, 192:320], x0[:, k, :], start=st, stop=sp)
            else:
                mm(z0[0:64, 0, :], w0[:, 0:64], x0, start=True, stop=True)
                mm(z0[:, 1, :], w0[:, 64:192], x0, start=True, stop=True)
                mm(z0[:, 2, :], w0[:, 192:320], x0, start=True, stop=True)
            nc.scalar.activation(gA, z0[:, 1, :], mybir.ActivationFunctionType.Sigmoid)
            nc.scalar.activation(gB, z0[:, 2, :], mybir.ActivationFunctionType.Sigmoid)
            nc.scalar.activation(sig, z0[0:64, 0, :], mybir.ActivationFunctionType.Sigmoid)
            nc.vector.tensor_mul(h0, z0[0:64, 0, :], sig)  # silu

            # A part (descending l): l=2 at rows 0:64, l=1 at rows 64:128
            if first:
                wl1, wl2 = W["W1_l1"], W["W1_l2"]
                a1 = lambda m: xA[0][:, m, :]
                a2 = lambda m: xA[1][:, m, :]
                ap1 = ap2 = 0
            else:
                wl2, wl1 = wA[0:64, :], wA[64:128, :]
                a2 = lambda m: xA[0:64, m, :]
                a1 = lambda m: xA[64:128, m, :]
                ap2, ap1 = 0, 64
            for mlo in (0, 3):
                nm = 3 if mlo == 0 else 2
                zA = zpool.tile([128, 3, BT], FP32, tag="zb")
                for j in range(nm):
                    m = mlo + j
                    mm(zA[0:64, j, :], wl2, a2(m), start=True, stop=True,
                       tile_position=(ap2, 0))
                    if m < 3:
                        mm(zA[64:128, j, :], wl1, a1(m), start=True, stop=True,
                           tile_position=(ap1, 64))
                if mlo == 0:
                    nc.vector.tensor_mul(
                        hA[:, 0:3, :], zA,
                        gA[:, :].unsqueeze(1).broadcast_to([128, 3, BT]))
                else:
                    nc.vector.tensor_mul(
                        hA[0:64, 3:5, :], zA[0:64, 0:2, :],
                        gA[0:64, :].unsqueeze(1).broadcast_to([64, 2, BT]))

            # B part (descending l): l=6 rows 0:32, l=5, l=4, l=3 rows 96:128
            if first:
                wof = {3: (W["W1_b34"], 0), 4: (W["W1_b34"], 64),
                       5: (W["W1_b56"], 0), 6: (W["W1_b56"], 64)}
                bsrc = {3: (xB[0], 0), 4: (xB[0], 64), 5: (xB[1], 0), 6: (xB[1], 64)}
                kw = 64
            else:
                wof = {l: (wB, 32 * (6 - l)) for l in (3, 4, 5, 6)}
                bsrc = {l: (xB, 32 * (6 - l)) for l in (3, 4, 5, 6)}
                kw = 32
            for mlo in (0, 3, 6, 9, 12):
                nm = min(3, 13 - mlo)
                zB = zpool.tile([128, 3, BT], FP32, tag="zb")
                phis = []
                for j in range(nm):
                    m = mlo + j
                    phi = 0
                    for l in (6, 5, 4, 3):
                        if m >= 2 * l + 1:
                            continue
                        wt, wr = wof[l]
                        xs, xr = bsrc[l]
                        pd = 32 * (6 - l)
                        phi = max(phi, pd + 32)
                        mm(zB[pd:pd + 32, j, :], wt[wr:wr + kw, :],
                           xs[xr:xr + kw, m, :], start=True, stop=True,
                           tile_position=(wr if first else xr, pd))
                    phis.append(phi)
                # gate in maximal rectangles of equal valid-partition prefix
                j = 0
                while j < nm:
                    k = j
                    while k < nm and phis[k] == phis[j]:
                        k += 1
                    phi = phis[j]
                    nc.vector.tensor_mul(
                        hB[0:phi, mlo + j:mlo + k, :], zB[0:phi, j:k, :],
                        gB[0:phi, :].unsqueeze(1).broadcast_to(
                            [phi, k - j, BT]))
                    j = k
            return h0, hA, hB

        if "1" not in stages:
            continue
        h0, hA, hB = layer(None, None, None, v0, (vA1, vA2), (vB34, vB56), True)
        if "2" in stages:
            h0, hA, hB = layer(W["W2_0"], W["W2_A"], W["W2_B"], h0, hA, hB, False)
        if "3" in stages:
            h0, hA, hB = layer(W["W3_0"], W["W3_A"], W["W3_B"], h0, hA, hB, False)
        if "4" not in stages:
            continue

        # ---------------- layer 4: accumulate into [49, BT] ----------------
        # layer 4: every matmul contracts a partition PREFIX (base 0) so the
        # whole 19-matmul accumulation chain shares PE row-group 0 and
        # serializes (concurrent same-address PSUM accumulation faults).
        z4 = z4pool.tile([D_OUT, BT], FP32, tag="z4")
        mm(z4, W["W4_0"], h0, start=True, stop=False, tile_position=(0, 0))
        for m in range(5):
            kp = 128 if m < 3 else 64      # [l2 | l1] prefix
            mm(z4, W["W4_A"][0:kp, m, :], hA[0:kp, m, :], start=False, stop=False,
               tile_position=(0, 0))
        for m in range(13):
            nvalid = sum(1 for l in (6, 5, 4, 3) if m < 2 * l + 1)
            kp = 32 * nvalid               # [l6 | l5 | l4 | l3] prefix
            mm(z4, W["W4_B"][0:kp, m, :], hB[0:kp, m, :],
               start=False, stop=(m == 12), tile_position=(0, 0))

        z4sb = opool.tile([D_OUT, BT], FP32, tag="z4sb")
        nc.vector.tensor_copy(z4sb, z4)
        for r in range(NR):
            z4t = z4pool.tile([128, D_OUT], FP32, tag="z4")
            nc.tensor.transpose(z4t, z4sb[:, r * 128:(r + 1) * 128], W["ident49"])
            outT = opool.tile([128, D_OUT], FP32, tag="outT")
            nc.vector.tensor_copy(outT, z4t)
            nc.sync.dma_start(out=out[t * BT + r * 128: t * BT + (r + 1) * 128, :],
                              in_=outT)


def _get_nc():
    if "nc" not in _BUILD:
        _BUILD["nc"] = _build_program()
    return _BUILD["nc"]


LAST_EXEC_NS = None
LAST_TRACE = None


def kernel(v_raw, w1, w2, w3, w4):
    global LAST_EXEC_NS, LAST_TRACE
    nc = _get_nc()
    wmap = _pack_weights(np.asarray(w1), np.asarray(w2), np.asarray(w3),
                         np.asarray(w4))
    v_raw = np.ascontiguousarray(np.asarray(v_raw, dtype=np.float32))
    in_maps = []
    for c in range(NCORES):
        m = dict(wmap)
        m["v"] = v_raw[c * BC:(c + 1) * BC]
        in_maps.append(m)
    res = bass_utils.run_bass_kernel_spmd(nc, in_maps, core_ids=list(range(NCORES)))
    if getattr(res, "exec_time_ns", None) is not None:
        LAST_EXEC_NS = res.exec_time_ns
        LAST_TRACE = getattr(res, "profile_json", None)
    outs = [res.results[c]["out"] for c in range(NCORES)]
    full = np.concatenate(outs, axis=0)            # [B, 49]
    return full.reshape(B_FULL, D_OUT, 1).astype(np.float32)

